# revision 45
# baseline (speedup 1.0000x reference)
"""Longformer self-attention Trainium2 kernel (8-core SPMD), v2.

Sharding: core c handles batch b = c//4 and heads [3*(c%4), 3*(c%4)+3).
Each core computes [4096, 192] (its 3 heads' output dims); the host
reassembles [2, 4096, 768].

v2 redesign (driven by the v1 hardware trace):
  - Head-pair packing: q/k/kg for heads (h0,h1) live in single [128, S]
    tiles (h0 rows 0:64, h1 rows 64:128); h2's q/k at base 0, kg/qg at
    base 64.  Projection W columns are packed into five 128-col chunks
    ([q01 | k01 | kg01 | q2,kg2 | k2]) so each PSUM chunk evacuates with
    one (or two) partition-aligned bias-add ops, split across DVE and
    the scalar engine.
  - Software pipeline: banded-attention work for block t is interleaved
    into the projection s-tile loop (sections), so the tensor engine
    never stalls on exp/mask latency and stays at full clock.
  - sel (global-key columns) for h0/h1 computed with one block-diagonal
    zero-padded [128, 64] stationary matmul (K=128) instead of two
    M=16/K=64 mms; h2 via a zero-padded [64, 64] stationary at PSUM
    partitions 64:128.  One exp over [96, 512] per s-tile (head blocks
    at 32-aligned partition offsets; exp(0)=1 gap rows are cancelled by
    vsel's zero rows).
  - sel PV contribution via a zero-padded [96, 65] v tile (K=96 matmul)
    instead of the pathological K=16 matmuls.
  - Normalization batched per unit: one reciprocal per (block, head),
    outputs assembled in [128, 2, 3, 64] SBUF tiles, one output DMA per
    (block, half) (31 + final global-row merge DMA).
  - exp() without max subtraction (logits are O(0.3)); masks are
    multiplicative {0,1} bf16 applied after exp, as in v1.
"""

import sys

sys.path.insert(0, "/opt/trn_rl_repo")

import numpy as np
import ml_dtypes

B, S, Dm, H, WIN, G, HD = 2, 4096, 768, 12, 256, 16, 64
HPC = 3            # heads per core
NCORES = 8
DPC = HPC * HD     # 192 output dims per core
NB = S // WIN      # 16 query blocks
NKC = S // 128     # 32 kpos chunks of 128
NST = 8            # s-tiles of 512
SCALE = 1.0 / 8.0  # 1/sqrt(64)

_CACHE = {}


def _mask_classes():
    """Multiplicative {0,1} masks in transposed-score orientation
    [kpos_local p, q_local r], applied to exp(scores).

    Chunk c of block t covers kpos = (2t-2+c)*128 + p, query i = 256t + r.
    Keep (1.0) iff the slot is band-valid and not a global key; global-key
    slots (kpos < G) and out-of-band slots contribute exactly 0 to the
    reference softmax (exp(-inf) / exp(x - 10000) both underflow to 0).
    """
    def build(t, c):
        p = np.arange(128)[:, None]
        r = np.arange(256)[None, :]
        kpos = (2 * t - 2 + c) * 128 + p
        i = 256 * t + r
        keep = (np.abs(kpos - i) <= WIN) & (kpos >= 0) & (kpos < S) & (kpos >= G)
        return keep.astype(np.float32)

    classes = {
        "t0c2": build(0, 2),
        "t1c0": build(1, 0),
        "c0": build(7, 0),
        "c1": build(7, 1),
        "c4": build(7, 4),
        "c5": build(7, 5),
    }
    lookup = {}
    for t in range(NB):
        cl, ch = _chunk_range(t)
        for c in range(cl, ch):
            if t == 0 and c == 2:
                mi = "t0c2"
            elif t == 1 and c == 0:
                mi = "t1c0"
            elif c == 0:
                mi = "c0"
            elif c == 1:
                mi = "c1"
            elif c == 4:
                mi = "c4"
            elif c == 5:
                mi = "c5"
            else:
                mi = None
            if mi is not None:
                assert np.array_equal(classes[mi], build(t, c)), (t, c, mi)
            else:
                assert np.all(build(t, c) == 1.0), (t, c)
            lookup[(t, c)] = mi
    return classes, lookup


def _chunk_range(t):
    if t == 0:
        return 2, 6
    if t == NB - 1:
        return 0, 4
    return 0, 6


def _patch_drain_and_barrier():
    """The walrus build in this container rejects >1 sync-wait on the CTRL
    (Drain) instruction that TileContext emits at exit ("Too many sync wait
    commands"). Split the waits: keep one on the drain, emit the rest as
    explicit single-sem wait_ge instructions on the sync engine before the
    barrier. Semantics preserved: all sems still quiesce before the
    sem-clear + barrier."""
    import concourse.tile as tile
    from concourse import mybir
    from concourse.vector_clock import ScopedClock

    if getattr(tile.TileContext, "_ant_drain_patch", False):
        return

    def _drain_and_barrier(self, tick_clock, wait_clock):
        nc = self.nc
        drain_inst = nc.sync.drain()
        wait_clock.add_sem_waits(
            drain_inst.ins, ScopedClock({None: tick_clock.global_clock})
        )
        si = drain_inst.ins.sync_info
        waits = list(si.on_wait) if si is not None else []
        if len(waits) > 1:
            drain_inst.ins.sync_info = mybir.SyncInfo(
                on_wait=[waits[0]], on_update=list(si.on_update)
            )
            allocated = self.sems.allocated()
            by_name = {}
            for key, sem in allocated.items():
                by_name[str(key)] = sem
                nm = getattr(sem, "name", None)
                if nm is not None:
                    by_name[str(nm)] = sem
            for w in waits[1:]:
                sem = by_name[w.ant_name]
                nc.sync.wait_ge(sem, w.wait_value)
        nc.all_engine_barrier()
        assert self.sems is not None
        popped = nc._tile_sem_poison_stack.pop()
        assert popped is self._sem_poison
        nc.clear_and_free_semaphores(list(self.sems.allocated().values()))
        nc.all_engine_barrier()

    tile.TileContext._drain_and_barrier = _drain_and_barrier
    tile.TileContext._ant_drain_patch = True


def _build_program():
    import concourse.bass as bass
    import concourse.tile as tile
    from concourse import bacc, mybir

    _patch_drain_and_barrier()

    f32 = mybir.dt.float32
    bf16 = mybir.dt.bfloat16
    AF = mybir.ActivationFunctionType
    AFexp = AF.Exp

    nc = bacc.Bacc(None)

    xT = nc.dram_tensor("xT", [Dm, S], bf16, kind="ExternalInput")
    # packed projection weights: 4 dc chunks
    #   dc0=[q_h0|q_h1] dc1=[k_h0|k_h1] dc2=[q_h2] dc3=[k_h2] (64 cols each)
    # (k biases are dropped entirely: a per-query constant shift of all key
    # logits cancels in the softmax; kg's full projection is eliminated via
    # gl = (Wkg qg)^T x, see wkgT/at_sb below — bkg likewise cancels.)
    Wqkkg = nc.dram_tensor("Wqkkg", [Dm, 384], bf16, kind="ExternalInput")
    Wvvg = nc.dram_tensor("Wvvg", [Dm, 2 * DPC], bf16, kind="ExternalInput")
    # qg chunks: [qg_h0|qg_h1], [zeros(64)|qg_h2]
    Wqg = nc.dram_tensor("Wqg", [Dm, 256], bf16, kind="ExternalInput")
    # Wkg transposed per head: [0:64,0]=h0, [64:128,0]=h1, [64:128,1]=h2
    WkgT = nc.dram_tensor("WkgT", [128, 2, Dm], bf16, kind="ExternalInput")
    # combined biases: cols 0:3 = q/k chunk biases, 3:5 = qg, 5:389 = v/vg
    # broadcast ([head, (v|vg), 64])
    b_all = nc.dram_tensor("b_all", [128, 389], f32, kind="ExternalInput")
    out_d = nc.dram_tensor("out", [S, DPC], f32, kind="ExternalOutput")

    classes, lookup = _mask_classes()
    mask_names = list(classes.keys())
    mask_np = np.stack([classes[k] for k in mask_names], axis=1)  # [128, 6, 256]
    masks_d = nc.inline_tensor(mask_np.astype(ml_dtypes.bfloat16), name="masks")
    midx = {k: i for i, k in enumerate(mask_names)}

    from contextlib import ExitStack

    with tile.TileContext(nc) as tc, ExitStack() as ctx:
        const = ctx.enter_context(tc.tile_pool(name="const", bufs=1))
        ph = ctx.enter_context(tc.tile_pool(name="ph", bufs=1))
        xpool = ctx.enter_context(tc.tile_pool(name="xpool", bufs=3))
        bx = ctx.enter_context(tc.tile_pool(name="bx", bufs=6))
        sbS = ctx.enter_context(tc.tile_pool(name="sbS", bufs=6))
        osbp = ctx.enter_context(tc.tile_pool(name="osbp", bufs=4))
        psA = ctx.enter_context(tc.tile_pool(name="psA", bufs=2, space="PSUM"))
        psP = ctx.enter_context(tc.tile_pool(name="psP", bufs=2, space="PSUM"))

        # ---- input loads, ordered by first-use time and spread across two
        # otherwise-idle queues so issue + descriptor-gen (~1.5us per DMA on
        # one sequencer) parallelizes.
        # sync queue: w6 halves, w6v, then per-section x prefetches.
        # scalar queue: biases (needed by the first evac at ~9us!), x-tile 0
        # halves, w6qg, wkgT, masks.
        ball_sb = const.tile([128, 389], f32, tag="ball", name="ball_sb")
        bqkkg_sb = ball_sb[:, 0:3]
        bqg_sb = ball_sb[:, 3:5]
        bvvg_sb = ball_sb[:, 5:389].rearrange("p (h g d) -> p h g d", h=HPC, g=2)
        w6 = const.tile([128, 6, 384], bf16, tag="w6", name="w6")
        xts = [None] * NST
        xts[0] = xpool.tile([128, 6, 512], bf16, tag="xt", name="xt0")
        for half in range(2):
            kcs = slice(3 * half, 3 * (half + 1))
            rsl = slice(384 * half, 384 * (half + 1))
            nc.sync.dma_start(
                out=w6[:, kcs, :],
                in_=Wqkkg[rsl, :].rearrange("(c p) d -> p c d", p=128),
            )
            nc.scalar.dma_start(
                out=xts[0][:, kcs, :],
                in_=xT[rsl, 0:512].rearrange("(c p) s -> p c s", p=128),
            )
            if half == 0:
                nc.sync.dma_start(out=ball_sb, in_=b_all[:])
        w6v = const.tile([128, 6, 2 * DPC], bf16, tag="w6v", name="w6v")
        nc.sync.dma_start(
            out=w6v, in_=Wvvg[:, :].rearrange("(c p) d -> p c d", p=128)
        )
        w6qg = const.tile([128, 6, 256], bf16, tag="w6qg", name="w6qg")
        nc.scalar.dma_start(
            out=w6qg, in_=Wqg[:, :].rearrange("(c p) d -> p c d", p=128)
        )
        wkgT = const.tile([128, 2, Dm], bf16, tag="wkgT", name="wkgT")
        nc.scalar.dma_start(out=wkgT, in_=WkgT[:])
        masks_sb = const.tile([128, 6, 256], bf16, tag="masks", name="masks_sb")
        nc.scalar.dma_start(out=masks_sb, in_=masks_d[:])

        # ---- persistent tensors ----
        qT01 = ph.tile([128, S], bf16, tag="qT01", name="qT01")
        kT01 = ph.tile([128, S], bf16, tag="kT01", name="kT01")
        # q2 rows 0:64, k2 rows 64:128 (single-op dc2 evacuation); k2 is
        # DMA-shifted to kT2 (base 0)
        qkT2f = ph.tile([128, S], bf16, tag="qkT2f", name="qkT2f")
        kT2 = ph.tile([64, S], bf16, tag="kT2", name="kT2")
        qgT01 = ph.tile([128, G], bf16, tag="qgT01", name="qgT01")
        qgT2f = ph.tile([128, G], bf16, tag="qgT2f", name="qgT2f")   # rows 64:128
        # at_sb[:, h, kc, :] = (Wkg_h qg_h)[dm-chunk kc, g] — replaces the
        # full kg projection (gl = at^T x)
        at_sb = ph.tile([128, HPC, 6, G], bf16, tag="at_sb", name="at_sb")
        # eg3[:, ci, h, :] = exp(gl) for s-chunk ci
        eg3 = ph.tile([128, NKC, HPC, G], bf16, tag="eg3", name="eg3")
        # selexp rows (32-aligned per head for partition-base legality):
        # 0:16 h0, 32:48 h1, 64:80 h2; gap rows hold exp(0)=1 garbage that is
        # cancelled by vsel's zero rows.
        selexp = ph.tile([96, S], bf16, tag="selexp", name="selexp")
        # block-diag stationaries (zero-padded): h0 cols 0:16, h1 cols 32:48
        seldiag = ph.tile([128, 64], bf16, tag="seldiag", name="seldiag")
        seldiag2 = ph.tile([64, 64], bf16, tag="seldiag2", name="seldiag2")
        # vsel[h]: rows 32h:32h+16 = [v_h[0:16] | 1], other rows zero
        vsel = ph.tile([96, HPC, HD + 1], bf16, tag="vsel", name="vsel")
        osb_t0 = ph.tile([128, 2, HPC, HD], f32, tag="osb_t0", name="osb_t0")
        # v/vg interleaved with ones column: [:, chunk, 2h+0, :] = v head h,
        # [:, chunk, 2h+1, :] = vg head h ([:, :, :, 64] = 1.0)
        vall = ph.tile([128, NKC, 2 * HPC, HD + 1], bf16, tag="vall", name="vall")
        nc.vector.memset(vall[:, :, :, HD : HD + 1], 1.0)
        nc.vector.memset(vsel[:], 0.0)
        nc.vector.memset(seldiag[:], 0.0)
        nc.vector.memset(seldiag2[:], 0.0)

        def mm(out, lhsT, rhs, start, stop):
            nc.tensor.matmul(out, lhsT, rhs, start=start, stop=stop)

        # per-head slices for band QK / sel / global
        def q_sl(h, csl):
            if h == 0:
                return qT01[0:64, csl]
            if h == 1:
                return qT01[64:128, csl]
            return qkT2f[0:64, csl]

        def k_sl(h, csl):
            if h == 0:
                return kT01[0:64, csl]
            if h == 1:
                return kT01[64:128, csl]
            return kT2[:, csl]

        def wkg_sl(h, ksl):
            if h == 0:
                return wkgT[0:64, 0, ksl]
            if h == 1:
                return wkgT[64:128, 0, ksl]
            return wkgT[64:128, 1, ksl]

        def qg_sl(h):
            if h == 0:
                return qgT01[0:64, :]
            if h == 1:
                return qgT01[64:128, :]
            return qgT2f[64:128, :]

        # ---------- emission helpers ----------
        units = [(t, h) for t in range(NB) for h in range(HPC)]
        state = {"qk": 0, "pv": 0, "prev_masks": None}
        osb_tiles = {}   # t -> [128, 2, HPC, 64] f32 tile

        def emit_qk(k):
            """Band QK for unit k + exp; mask muls for the previous unit.

            The previous unit's mask muls are flushed first (they wait on its
            exp; emitting them after this point's proj-evacs avoids
            head-of-line blocking on DVE).
            """
            flush_masks()
            if k >= len(units):
                return
            t, h = units[k]
            cl, ch = _chunk_range(t)
            qsl = slice(256 * t, 256 * (t + 1))
            sc = psA.tile([128, 6, 256], f32, tag="sc", name="sc")
            for c in range(cl, ch):
                j = 2 * t - 2 + c
                mm(sc[:, c, :], k_sl(h, slice(128 * j, 128 * (j + 1))), q_sl(h, qsl),
                   True, True)
            bexp = bx.tile([128, 6, 256], bf16, tag="bexp", name="bexp")
            nc.scalar.activation(out=bexp[:, cl:ch, :], in_=sc[:, cl:ch, :], func=AFexp)
            state["bexp" + str(k)] = bexp
            state["prev_masks"] = (t, h, bexp)

        def emit_pv(k):
            if k >= len(units):
                return
            t, h = units[k]
            cl, ch = _chunk_range(t)
            bexp = state.pop("bexp" + str(k))
            at = psP.tile([128, 2, HD + 1], f32, tag="small", name="at")
            if t == 0:
                osb3 = osb_t0
            elif t not in osb_tiles:
                osb3 = osb_tiles[t] = osbp.tile(
                    [128, 2, HPC, HD], f32, tag="osb", name="osb"
                )
            else:
                osb3 = osb_tiles[t]
            for half in range(2):
                hs = slice(128 * half, 128 * (half + 1))
                for c in range(cl, ch):
                    j = 2 * t - 2 + c
                    mm(at[:, half, :], bexp[:, c, hs], vall[:, j, 2 * h, :],
                       c == cl, False)
                q0 = 256 * t + 128 * half
                mm(at[:, half, :], selexp[:, q0 : q0 + 128], vsel[:, h, :],
                   False, True)
            rec = sbS.tile([128, 2, 1], f32, tag="rec", name="rec")
            nc.vector.reciprocal(rec, at[:, :, HD : HD + 1])
            for half in range(2):
                nc.vector.tensor_scalar_mul(
                    osb3[:, half, h, :], at[:, half, 0:HD], rec[:, half, :]
                )
            if h == HPC - 1:
                for half in range(2):
                    if t == 0 and half == 0:
                        continue  # deferred: rows 0:16 come from the global path
                    q0 = 256 * t + 128 * half
                    nc.sync.dma_start(
                        out=out_d[q0 : q0 + 128, :], in_=osb3[:, half, :, :]
                    )
                if t != 0:
                    del osb_tiles[t]

        def band_step():
            """One pipeline step: QK(qk_i) (+exp), PV two+ units behind."""
            emit_qk(state["qk"])
            state["qk"] += 1
            if state["pv"] >= 0 and state["pv"] <= state["qk"] - 3:
                emit_pv(state["pv"])
                state["pv"] += 1

        def flush_masks():
            pm = state["prev_masks"]
            if pm is not None:
                t, h, bexp = pm
                cl, ch = _chunk_range(t)
                for c in range(cl, ch):
                    mi = lookup[(t, c)]
                    if mi is not None:
                        nc.vector.tensor_mul(
                            bexp[:, c, :], bexp[:, c, :], masks_sb[:, midx[mi], :]
                        )
                state["prev_masks"] = None

        def pv_drain_step(force=False):
            flush_masks()
            lim = state["qk"] - (0 if force else 2)
            if state["pv"] < min(lim, len(units)):
                emit_pv(state["pv"])
                state["pv"] += 1

        def emit_sel(st):
            ssl = slice(512 * st, 512 * (st + 1))
            sps = psP.tile([128, 512], f32, tag="small", name="sps")
            mm(sps[0:64, :], seldiag, qT01[:, ssl], True, True)
            mm(sps[64:128, :], seldiag2, qkT2f[0:64, ssl], True, True)
            nc.scalar.activation(out=selexp[:, ssl], in_=sps[0:96, :], func=AFexp)

        # dc chunk evacuation (k is bias-free: its logit shift cancels in
        # the softmax, so cols 1/3 of b_qkkg are zeros)
        def evac_dc(dc, ps, ssl, rows):
            if dc == 0:
                nc.vector.tensor_scalar_add(qT01[:, ssl], ps[:, :],
                                            bqkkg_sb[:, 0:1])
            elif dc == 1:
                nc.scalar.add(kT01[:, ssl], ps[:, :], bqkkg_sb[:, 1:2])
            else:
                # dc2 = [q_h2; k_h2]: k lands at partitions 64:128 and is
                # shifted to kT2 (base 0) with an SBUF->SBUF DMA, so band QK
                # h2 keeps partition-aligned operands
                nc.vector.tensor_scalar_add(
                    qkT2f[:, ssl], ps[:, :], bqkkg_sb[:, 2:3]
                )
                nc.sync.dma_start(out=kT2[:, ssl], in_=qkT2f[64:128, ssl])

        def emit_gl(xt, sc_i, ci):
            """gl[s, g] = x^T at for one 128-wide s-chunk, all heads batched
            per matmul (rhs [128, 3, 16] strided view of at_sb)."""
            msl = slice(128 * sc_i, 128 * (sc_i + 1))
            psgl = psP.tile([128, HPC, G], f32, tag="small", name="psgl")
            for kc in range(6):
                mm(psgl, xt[:, kc, msl], at_sb[:, :, kc, :], kc == 0, kc == 5)
            nc.scalar.activation(out=eg3[:, ci, :, :], in_=psgl, func=AFexp)

        # ---------- main section loop ----------
        for st in range(NST):
            ssl = slice(512 * st, 512 * (st + 1))
            xt = xts[st]

            # q/k projection chains (3 dc chunks); the next x-tile prefetch is
            # issued at dc1 so the dc2 kT2-shift DMA shares the queue fairly
            for dc in range(3):
                d0, d1 = 128 * dc, 128 * (dc + 1)
                ps = psP.tile([128, 512], f32, tag="small", name="psqk")
                for kc in range(6):
                    mm(ps, w6[:, kc, d0:d1], xt[:, kc, :], kc == 0, kc == 5)
                evac_dc(dc, ps, ssl, 128)
                if dc == 1 and st + 1 < NST:
                    nxt = xpool.tile([128, 6, 512], bf16, tag="xt", name="xt")
                    xts[st + 1] = nxt
                    nc.sync.dma_start(
                        out=nxt,
                        in_=xT[:, 512 * (st + 1) : 512 * (st + 2)].rearrange(
                            "(c p) s -> p c s", p=128
                        ),
                    )
                if st >= 1:
                    band_step()

            if st == 0:
                # qg projections (need only xt0 cols 0:16)
                pq1 = psP.tile([128, G], f32, tag="small", name="pq1")
                for kc in range(6):
                    mm(pq1, w6qg[:, kc, 0:128], xt[:, kc, 0:G], kc == 0, kc == 5)
                nc.scalar.add(qgT01, pq1, bqg_sb[:, 0:1])
                pq2 = psP.tile([128, G], f32, tag="small", name="pq2")
                for kc in range(6):
                    mm(pq2, w6qg[:, kc, 128:256], xt[:, kc, 0:G], kc == 0, kc == 5)
                nc.vector.tensor_scalar_add(
                    qgT2f[64:128, :], pq2[64:128, :], bqg_sb[64:128, 1:2]
                )
                # at = Wkg qg per head (the rank-16 substitute for the full
                # kg projection)
                for h in range(HPC):
                    pAT = psP.tile([128, 6, G], f32, tag="small", name="pAT")
                    for kc in range(6):
                        mm(pAT[:, kc, :], wkg_sl(h, slice(128 * kc, 128 * (kc + 1))),
                           qg_sl(h), True, True)
                    nc.vector.tensor_copy(out=at_sb[:, h, :, :], in_=pAT)

            # vvg chains (4 s-chunks, x stationary) + gl + band steps
            for sc_i in range(4):
                ci = 4 * st + sc_i
                msl = slice(128 * sc_i, 128 * (sc_i + 1))
                psv = psP.tile([128, 2 * DPC], f32, tag="small", name="psv")
                for kc in range(6):
                    mm(psv, xt[:, kc, msl], w6v[:, kc, :], kc == 0, kc == 5)
                src = bass.AP(
                    tensor=psv.tensor,
                    offset=psv.offset,
                    ap=[psv.ap[0], [HD, HPC], [DPC, 2], [1, HD]],
                )
                dst = vall[:, ci, :, 0:HD].rearrange("p (h g) d -> p h g d", h=HPC)
                nc.vector.tensor_add(dst, src, bvvg_sb)
                # gl for sc 2 and 3 are emitted before psv2 so the section's
                # last PSUM-slot tiles are DVE-consumed (v-adds), keeping the
                # next section's first chain from waiting on the Act backlog
                if sc_i == 0 or sc_i == 1:
                    emit_gl(xt, sc_i, ci)
                elif sc_i == 2:
                    pass  # gl2+gl3 already emitted below at sc_i==1
                if st >= 1:
                    if sc_i == 0:
                        emit_sel(st)
                        band_step()
                    elif sc_i == 1:
                        pv_drain_step()
                    else:
                        band_step()
                if sc_i == 1:
                    emit_gl(xt, 2, 4 * st + 2)
                    emit_gl(xt, 3, 4 * st + 3)

            if st == 0:
                # build seldiag (kT01/kT2 cols 0:16 from st0) + vsel, then sel
                # for s-tile 0 — needed by the first PVs early in section 1
                nc.vector.tensor_copy(out=seldiag[0:64, 0:G], in_=kT01[0:64, 0:G])
                nc.vector.tensor_copy(
                    out=seldiag[64:128, 2 * G : 3 * G], in_=kT01[64:128, 0:G]
                )
                nc.vector.tensor_copy(out=seldiag2[:, 0:G], in_=kT2[:, 0:G])
                for h in range(HPC):
                    nc.sync.dma_start(
                        out=vsel[32 * h : 32 * h + G, h, 0:HD],
                        in_=vall[0:G, 0, 2 * h, 0:HD],
                    )
                    nc.vector.memset(
                        vsel[32 * h : 32 * h + G, h, HD : HD + 1], 1.0
                    )
                emit_sel(0)

        # ---------- tail: blocks 14, 15 + global-token rows ----------
        for h in range(HPC):
            band_step()
            ops = psP.tile([G, HD + 1], f32, tag="small", name="ops")
            for c in range(NKC):
                mm(ops, eg3[:, c, h, :], vall[:, c, 2 * h + 1, :],
                   c == 0, c == NKC - 1)
            recg = sbS.tile([G, 1], f32, tag="recg", name="recg")
            nc.vector.reciprocal(recg, ops[:, HD : HD + 1])
            # overwrite rows 0:16 of block 0's half-0 output with the
            # global-token-row results (reference semantics)
            nc.vector.tensor_scalar_mul(
                osb_t0[0:G, 0, h, :], ops[:, 0:HD], recg
            )
            band_step()
        nc.sync.dma_start(out=out_d[0:128, :], in_=osb_t0[:, 0, :, :])
        while state["qk"] < len(units):
            band_step()
        while state["pv"] < len(units):
            pv_drain_step(force=True)

    return nc


def _get_program():
    if "nc" not in _CACHE:
        nc = _build_program()
        nc.finalize()
        _CACHE["nc"] = nc
    return _CACHE["nc"]


def _prep_in_maps(hidden_states, Wq, bq, Wk, bk, Wv, bv, Wqg, bqg, Wkg, bkg, Wvg, bvg):
    hs = np.asarray(hidden_states, dtype=np.float32)
    f32 = np.float32
    bfl = ml_dtypes.bfloat16
    in_maps = []
    for c in range(NCORES):
        b = c // 4
        h0 = 3 * (c % 4)

        def wcol(Wm, h, scale=1.0):
            return np.asarray(Wm)[:, HD * (h0 + h) : HD * (h0 + h + 1)] * scale

        def bcol(v, h, scale=1.0):
            return np.asarray(v)[HD * (h0 + h) : HD * (h0 + h + 1)] * scale

        cols = slice(HD * h0, HD * (h0 + 3))
        # k biases are dropped: a per-query constant shift of every key
        # logit (q . bk) cancels in the softmax; likewise bkg for the
        # global rows (softmax over s).
        Wqkkg = np.concatenate(
            [
                wcol(Wq, 0, SCALE), wcol(Wq, 1, SCALE),
                wcol(Wk, 0), wcol(Wk, 1),
                wcol(Wq, 2, SCALE), wcol(Wk, 2),
            ],
            axis=1,
        )
        b_qkkg = np.stack(
            [
                np.concatenate([bcol(bq, 0, SCALE), bcol(bq, 1, SCALE)]),
                np.zeros(128, f32),
                np.concatenate([bcol(bq, 2, SCALE), np.zeros(HD, f32)]),
            ],
            axis=1,
        ).astype(f32)
        # WkgT[0:64, 0] = Wkg_h0^T, [64:128, 0] = h1^T, [64:128, 1] = h2^T
        WkgT_p = np.zeros((128, 2, Dm), f32)
        WkgT_p[0:64, 0] = wcol(Wkg, 0).T
        WkgT_p[64:128, 0] = wcol(Wkg, 1).T
        WkgT_p[64:128, 1] = wcol(Wkg, 2).T
        Wqg_p = np.concatenate(
            [
                wcol(Wqg, 0, SCALE), wcol(Wqg, 1, SCALE),
                np.zeros((Dm, HD), f32), wcol(Wqg, 2, SCALE),
            ],
            axis=1,
        )
        b_qg = np.stack(
            [
                np.concatenate([bcol(bqg, 0, SCALE), bcol(bqg, 1, SCALE)]),
                np.concatenate([np.zeros(HD, f32), bcol(bqg, 2, SCALE)]),
            ],
            axis=1,
        ).astype(f32)
        bvvg = np.stack(
            [
                np.asarray(bv)[cols].reshape(HPC, HD),
                np.asarray(bvg)[cols].reshape(HPC, HD),
            ],
            axis=1,
        ).astype(f32)  # [3, 2, 64]
        in_maps.append(
            {
                "xT": np.ascontiguousarray(hs[b].T).astype(bfl),
                "Wqkkg": np.ascontiguousarray(Wqkkg).astype(bfl),
                "WkgT": np.ascontiguousarray(WkgT_p).astype(bfl),
                "Wvvg": np.ascontiguousarray(np.concatenate(
                    [np.asarray(Wv)[:, cols], np.asarray(Wvg)[:, cols]], axis=1
                )).astype(bfl),
                "Wqg": np.ascontiguousarray(Wqg_p).astype(bfl),
                "b_all": np.ascontiguousarray(np.concatenate(
                    [
                        b_qkkg,
                        b_qg,
                        np.broadcast_to(
                            bvvg.reshape(-1)[None], (128, HPC * 2 * HD)
                        ),
                    ],
                    axis=1,
                ).astype(f32)),
            }
        )
    return in_maps


def kernel(
    hidden_states,
    Wq,
    bq,
    Wk,
    bk,
    Wv,
    bv,
    Wqg,
    bqg,
    Wkg,
    bkg,
    Wvg,
    bvg,
    n_global,
):
    from concourse.bass_utils import run_bass_kernel_spmd

    assert int(n_global) == G
    nc = _get_program()
    in_maps = _prep_in_maps(
        hidden_states, Wq, bq, Wk, bk, Wv, bv, Wqg, bqg, Wkg, bkg, Wvg, bvg
    )
    res = run_bass_kernel_spmd(nc, in_maps, list(range(NCORES)))
    out = np.zeros((B, S, Dm), np.float32)
    for c in range(NCORES):
        b = c // 4
        cols = slice(HD * 3 * (c % 4), HD * (3 * (c % 4) + 3))
        out[b, :, cols] = res.results[c]["out"]
    return out
